# revision 2
# baseline (speedup 1.0000x reference)
"""CycleFC forward on 8 Trainium2 NeuronCores.

Problem: x [64, 256, 56, 56] f32, weight [256, 256], bias [256].
  out[b,o,h,w] = sum_c weight[o,c] * x[b,c,h,w+s_c] + bias[o]
  with s_c = (c+3) % 7 - 3 and zero padding outside [0, W).

Strategy:
  - Data-parallel over batch: 8 batches per core.
  - The per-channel shift is absorbed into the DMA load offset: the host
    pre-pads each (c, h) row to width 62 (3 zeros each side) so channel c's
    whole padded [56, 62] plane is loaded as ONE contiguous run starting at
    element (3 + s_c).  After that, every channel's SBUF row holds
    xs[c, h*62 + w] = x[c, h, w + s_c] (zeros off the edge), so a plain
    matmul with a strided rhs access pattern ([h-rows, 62-stride] x [56, 1])
    computes the shifted 1x1 conv exactly.  Channels are host-permuted so
    that each shift group is a contiguous partition range (weights permuted
    to match along the contraction dim only; output channel order is
    untouched).
"""

import numpy as np

C = 256
H = 56
W = 56
B_PER_CORE = 8
N_CORES = 8
K = 7
WP = 62           # padded row width
PLANE = H * WP    # 3472 padded elements per (b, c) plane
LOAD = (H - 1) * WP + W   # 3466: elements DMAed per channel (covers max AP read)
HW = H * W        # 3136
ROWS_PER_MM = 8   # h-rows per matmul -> free dim 448 (<=512 fp32 PSUM bank)
NT = H // ROWS_PER_MM  # 7 n-tiles
FREE = ROWS_PER_MM * W  # 448

# shift for channel group j (channels c with c % 7 == j, permuted contiguous)
_SHIFTS = [(j + 3) % K - K // 2 for j in range(K)]          # [0,1,2,3,-3,-2,-1]
_GROUP_SIZES = [len(range(j, C, K)) for j in range(K)]       # [37,37,37,37,36,36,36]
_GROUP_STARTS = np.cumsum([0] + _GROUP_SIZES).tolist()       # [0,37,74,111,148,184,220,256]


def _chunk_segments():
    """Per 128-partition contraction chunk: list of (local_lo, local_hi, shift)."""
    segs = [[], []]
    for j in range(K):
        glo, ghi = _GROUP_STARTS[j], _GROUP_STARTS[j + 1]
        for chunk in range(2):
            c0, c1 = chunk * 128, chunk * 128 + 128
            lo, hi = max(glo, c0), min(ghi, c1)
            if lo < hi:
                segs[chunk].append((lo - c0, hi - c0, _SHIFTS[j]))
    return segs


def build_nc(mm_dtype="float32", x_bufs=4, o_bufs=3, ps_bufs=8):
    """Build the single-core Bass program (SPMD across 8 cores)."""
    import concourse.mybir as mybir
    import concourse.tile as tile
    from concourse import bacc

    f32 = mybir.dt.float32
    mmdt = getattr(mybir.dt, mm_dtype)

    nc = bacc.Bacc("TRN2", target_bir_lowering=False, debug=False,
                   enable_asserts=False)
    xp = nc.dram_tensor("xp", [B_PER_CORE, C, PLANE], f32,
                        kind="ExternalInput").ap()
    wT = nc.dram_tensor("wT", [C, C], f32, kind="ExternalInput").ap()
    biasT = nc.dram_tensor("biasT", [128, 2], f32, kind="ExternalInput").ap()
    out = nc.dram_tensor("out", [B_PER_CORE, C, HW], f32,
                         kind="ExternalOutput").ap()

    segs = _chunk_segments()

    with tile.TileContext(nc) as tc:
        with (
            tc.tile_pool(name="w", bufs=1) as wpool,
            tc.tile_pool(name="x", bufs=x_bufs) as xpool,
            tc.tile_pool(name="o", bufs=o_bufs) as opool,
            tc.tile_pool(name="ps", bufs=ps_bufs, space="PSUM") as pspool,
        ):
            w0 = wpool.tile([128, C], f32, tag="w0")
            w1 = wpool.tile([128, C], f32, tag="w1")
            nc.sync.dma_start(w0[:], wT[0:128, :])
            nc.sync.dma_start(w1[:], wT[128:256, :])
            bt = wpool.tile([128, 2], f32, tag="bias")
            nc.sync.dma_start(bt[:], biasT[:])

            for b in range(B_PER_CORE):
                xs = []
                for chunk in range(2):
                    xt = xpool.tile([128, PLANE], f32, tag="x",
                                    name=f"x_b{b}c{chunk}")
                    for (lo, hi, s) in segs[chunk]:
                        off = 3 + s
                        nc.sync.dma_start(
                            xt[lo:hi, 0:LOAD],
                            xp[b, chunk * 128 + lo:chunk * 128 + hi,
                               off:off + LOAD])
                    xs.append(xt)
                rhs_views = [
                    x[:].rearrange("p (h w) -> p h w", w=WP) for x in xs
                ]
                for o in range(2):
                    osb = opool.tile([128, HW], f32, tag="o",
                                     name=f"o_b{b}o{o}")
                    for t in range(NT):
                        ps = pspool.tile([128, FREE], f32, tag="ps",
                                         name=f"ps_b{b}o{o}t{t}")
                        for chunk in range(2):
                            rhs = rhs_views[chunk][
                                :, t * ROWS_PER_MM:(t + 1) * ROWS_PER_MM, 0:W]
                            lhsT = (w0 if chunk == 0 else w1)[
                                :, o * 128:(o + 1) * 128]
                            nc.tensor.matmul(
                                ps[:],
                                lhsT.bitcast(mmdt) if mm_dtype != "float32" else lhsT,
                                rhs.bitcast(mmdt) if mm_dtype != "float32" else rhs,
                                start=(chunk == 0), stop=(chunk == 1))
                        nc.vector.tensor_scalar(
                            out=osb[:, t * FREE:(t + 1) * FREE],
                            in0=ps[:],
                            scalar1=bt[:, o:o + 1],
                            scalar2=None,
                            op0=mybir.AluOpType.add)
                    nc.sync.dma_start(out[b, o * 128:(o + 1) * 128, :], osb[:])
    nc.compile()
    return nc


def _host_prep(x, weight, bias):
    perm = np.concatenate([np.arange(j, C, K) for j in range(K)])
    xp = np.zeros((x.shape[0], C, H, WP), dtype=np.float32)
    xp[:, :, :, 3:3 + W] = x[:, perm]
    xp = xp.reshape(x.shape[0], C, PLANE)
    wT = np.ascontiguousarray(weight[:, perm].T.astype(np.float32))
    biasT = np.ascontiguousarray(bias.astype(np.float32).reshape(2, 128).T)
    return xp, wT, biasT


_NC_CACHE = {}


def _get_nc(mm_dtype="float32"):
    if mm_dtype not in _NC_CACHE:
        _NC_CACHE[mm_dtype] = build_nc(mm_dtype)
    return _NC_CACHE[mm_dtype]


def kernel(x, weight, bias, mm_dtype="float32"):
    from concourse.bass_utils import run_bass_kernel_spmd

    x = np.asarray(x, dtype=np.float32)
    weight = np.asarray(weight, dtype=np.float32)
    bias = np.asarray(bias, dtype=np.float32)
    B = x.shape[0]
    assert B == B_PER_CORE * N_CORES and x.shape[1:] == (C, H, W)

    nc = _get_nc(mm_dtype)
    xp, wT, biasT = _host_prep(x, weight, bias)
    in_maps = [
        {"xp": np.ascontiguousarray(xp[c * B_PER_CORE:(c + 1) * B_PER_CORE]),
         "wT": wT, "biasT": biasT}
        for c in range(N_CORES)
    ]
    res = run_bass_kernel_spmd(nc, in_maps, core_ids=list(range(N_CORES)))
    out = np.concatenate(
        [r["out"].reshape(B_PER_CORE, C, H, W) for r in res.results], axis=0)
    return out


# revision 13
# speedup vs baseline: 2.0587x; 2.0587x over previous
"""CycleFC forward on 8 Trainium2 NeuronCores.

Problem: x [64, 256, 56, 56] f32, weight [256, 256], bias [256].
  out[b,o,h,w] = sum_c weight[o,c] * x[b,c,h,w+s_c] + bias[o]
  with s_c = (c+3) % 7 - 3 and zero padding outside [0, W).

Strategy:
  - Data-parallel over batch: 8 batches per core.
  - The per-channel shift is absorbed into the DMA load offset: the host
    pads each (c, h) row to stride 59 ([3 zeros][56 data]; a row's
    right-shift reads land in the next row's left-pad zeros) so channel c's
    whole padded plane is loaded as ONE contiguous run starting at element
    (3 + s_c).  After that, every channel's SBUF row holds
    xs[c, h*59 + w] = x[c, h, w + s_c] (zeros off the edge), so a plain
    matmul with a strided rhs access pattern ([h-rows, 59-stride] x [56, 1])
    computes the shifted 1x1 conv exactly.  Channels are host-permuted so
    that each shift group is a contiguous partition range (weights permuted
    to match along the contraction dim only; output channel order is
    untouched).
  - matmul in float32r (1 cycle/row vs 4 for float32); inputs keep fp32
    bits, PSUM accumulates fp32.  rel err vs fp32 reference ~1.4e-4.
  - Input loads on the SP HWDGE ring, output stores on the ACT HWDGE ring
    (separate FIFOs - stores gated on compute must not head-of-line-block
    the prefetch loads).
"""

import contextlib

import numpy as np

C = 256
H = 56
W = 56
B_PER_CORE = 8
N_CORES = 8
K = 7
WP = 59           # padded row stride ([3 zeros][56 data] per row; row h's
                  # right-pad reads land in row h+1's left-pad zeros)
PLANE = H * WP + (62 - WP)   # DRAM plane: + tail zeros for the max shift
TILE_PLANE = H * WP          # SBUF tile free size (divisible by WP)
LOAD = (H - 1) * WP + W      # elements DMAed per channel (covers max AP read)
HW = H * W        # 3136
ROWS_PER_MM = 8   # h-rows per matmul -> free dim 448 (<=512 fp32 PSUM bank)
NT = H // ROWS_PER_MM  # 7 n-tiles
FREE = ROWS_PER_MM * W  # 448

# shift for channel group j (channels c with c % 7 == j, permuted contiguous)
_SHIFTS = [(j + 3) % K - K // 2 for j in range(K)]          # [0,1,2,3,-3,-2,-1]
_GROUP_SIZES = [len(range(j, C, K)) for j in range(K)]       # [37,37,37,37,36,36,36]
_GROUP_STARTS = np.cumsum([0] + _GROUP_SIZES).tolist()


def _chunk_segments():
    """Per 128-partition contraction chunk: list of (local_lo, local_hi, shift)."""
    segs = [[], []]
    for j in range(K):
        glo, ghi = _GROUP_STARTS[j], _GROUP_STARTS[j + 1]
        for chunk in range(2):
            c0, c1 = chunk * 128, chunk * 128 + 128
            lo, hi = max(glo, c0), min(ghi, c1)
            if lo < hi:
                segs[chunk].append((lo - c0, hi - c0, _SHIFTS[j]))
    return segs


def build_nc(mm_dtype="float32r", x_bufs=4, o_bufs=3, ps_bufs=8,
             store_eng="scalar", reps=1, loop_reps=0, dma_only=0, tiny_loop=0):
    """Build the single-core Bass program (SPMD across 8 cores).

    reps/loop_reps/dma_only/tiny_loop are dev-only knobs for timing probes.
    """
    import concourse.mybir as mybir
    import concourse.tile as tile
    from concourse import bacc

    f32 = mybir.dt.float32
    mmdt = getattr(mybir.dt, mm_dtype)

    nc = bacc.Bacc("TRN2", target_bir_lowering=False, debug=False,
                   enable_asserts=False)
    xp = nc.dram_tensor("xp", [B_PER_CORE, C, PLANE], mmdt,
                        kind="ExternalInput").ap()
    wT = nc.dram_tensor("wT", [C, C], mmdt, kind="ExternalInput").ap()
    biasT = nc.dram_tensor("biasT", [128, 2], f32, kind="ExternalInput").ap()
    out = nc.dram_tensor("out", [B_PER_CORE, C, HW], f32,
                         kind="ExternalOutput").ap()

    segs = _chunk_segments()
    store = getattr(nc, store_eng)

    def one_pass(rep, xpool, opool, pspool, w0, w1, bt):
        for b in range(B_PER_CORE):
            xs = []
            for chunk in range(2):
                xt = xpool.tile([128, TILE_PLANE], mmdt, tag="x",
                                name=f"x_r{rep}b{b}c{chunk}")
                for (lo, hi, s) in segs[chunk]:
                    off = 3 + s
                    nc.sync.dma_start(
                        xt[lo:hi, 0:LOAD],
                        xp[b, chunk * 128 + lo:chunk * 128 + hi,
                           off:off + LOAD])
                xs.append(xt)
            rhs_views = [x[:].rearrange("p (h w) -> p h w", w=WP) for x in xs]
            for o in range(2):
                osb = opool.tile([128, HW], f32, tag="o",
                                 name=f"o_r{rep}b{b}o{o}")
                if dma_only:
                    nc.vector.memset(osb[:, 0:8], 0.0)
                    store.dma_start(out[b, o * 128:(o + 1) * 128, :], osb[:])
                    continue
                for t in range(NT):
                    ps = pspool.tile([128, FREE], f32, tag="ps",
                                     name=f"ps_r{rep}b{b}o{o}t{t}")
                    for chunk in range(2):
                        rhs = rhs_views[chunk][
                            :, t * ROWS_PER_MM:(t + 1) * ROWS_PER_MM, 0:W]
                        lhsT = (w0 if chunk == 0 else w1)[
                            :, o * 128:(o + 1) * 128]
                        nc.tensor.matmul(ps[:], lhsT, rhs,
                                         start=(chunk == 0), stop=(chunk == 1))
                    nc.vector.tensor_scalar(
                        out=osb[:, t * FREE:(t + 1) * FREE],
                        in0=ps[:],
                        scalar1=bt[:, o:o + 1],
                        scalar2=None,
                        op0=mybir.AluOpType.add)
                store.dma_start(out[b, o * 128:(o + 1) * 128, :], osb[:])

    with tile.TileContext(nc) as tc:
        with (
            tc.tile_pool(name="w", bufs=1) as wpool,
            tc.tile_pool(name="x", bufs=x_bufs) as xpool,
            tc.tile_pool(name="o", bufs=o_bufs) as opool,
            tc.tile_pool(name="ps", bufs=ps_bufs, space="PSUM") as pspool,
        ):
            w0 = wpool.tile([128, C], mmdt, tag="w0")
            w1 = wpool.tile([128, C], mmdt, tag="w1")
            nc.sync.dma_start(w0[:], wT[0:128, :])
            nc.sync.dma_start(w1[:], wT[128:256, :])
            bt = wpool.tile([128, 2], f32, tag="bias")
            nc.sync.dma_start(bt[:], biasT[:])

            loop_cm = tc.For_i(0, loop_reps, 1) if loop_reps else \
                contextlib.nullcontext()
            with loop_cm:
                if tiny_loop:
                    xt = xpool.tile([128, 512], mmdt, tag="x", name="tiny")
                    nc.sync.dma_start(xt[:], xp[0, 0:128, 0:512])
                    store.dma_start(out[0, 0:128, 0:512],
                                    xt[:].bitcast(f32))
                else:
                    for rep in range(reps):
                        one_pass(rep, xpool, opool, pspool, w0, w1, bt)
    nc.compile()
    return nc


def _host_prep(x, weight, bias):
    perm = np.concatenate([np.arange(j, C, K) for j in range(K)])
    xp = np.zeros((x.shape[0], C, PLANE), dtype=np.float32)
    xp[:, :, :H * WP].reshape(x.shape[0], C, H, WP)[:, :, :, 3:3 + W] = x[:, perm]
    wT = np.ascontiguousarray(weight[:, perm].T.astype(np.float32))
    biasT = np.ascontiguousarray(bias.astype(np.float32).reshape(2, 128).T)
    return xp, wT, biasT


_NC_CACHE = {}


def _get_nc(mm_dtype="float32r"):
    if mm_dtype not in _NC_CACHE:
        _NC_CACHE[mm_dtype] = build_nc(mm_dtype)
    return _NC_CACHE[mm_dtype]


def kernel(x, weight, bias, mm_dtype="float32r"):
    from concourse.bass_utils import run_bass_kernel_spmd

    x = np.asarray(x, dtype=np.float32)
    weight = np.asarray(weight, dtype=np.float32)
    bias = np.asarray(bias, dtype=np.float32)
    B = x.shape[0]
    assert B == B_PER_CORE * N_CORES and x.shape[1:] == (C, H, W)

    nc = _get_nc(mm_dtype)
    xp, wT, biasT = _host_prep(x, weight, bias)
    in_maps = [
        {"xp": np.ascontiguousarray(xp[c * B_PER_CORE:(c + 1) * B_PER_CORE]),
         "wT": wT, "biasT": biasT}
        for c in range(N_CORES)
    ]
    res = run_bass_kernel_spmd(nc, in_maps, core_ids=list(range(N_CORES)))
    out = np.concatenate(
        [r["out"].reshape(B_PER_CORE, C, H, W) for r in res.results], axis=0)
    return out
